# revision 93
# baseline (speedup 1.0000x reference)
"""Blockwise-parallel transformer layer on 8 TRN2 NeuronCores.

Sharding: by kv-head (the reference's einsum ties kv-head to seq pos mod 16).
Core c owns heads {2c, 2c+1} and the 256 seq rows n with n%16 in {2c, 2c+1}.
K/V projections therefore only need the 128-wide Wk/Wv column slice for the
core's two heads (8x less replicated GEMM work than seq-sharding, and no
collectives).

Attention runs in fp8 (e4m3) DoubleRow on the PE (2x bf16 rate): QKV
projections (weights prescaled x32, the 1/32 rides the psum copy-out, q/k/v
requantized to fp8), pass-A q-major scores for the per-kv-block max (DVE
reduce over a stride-2 sample — the block-scale tolerance absorbs the bias),
and pass-B kc-major scores where the indicator rows (streamed from DRAM)
broadcast -max straight into the psum; exp applies the 1/sqrt(HD) scale and
writes fp8, so num/den is a DoubleRow matmul over kv-block PAIRS with a
ones-column in V providing den.  V is projected directly in kc-major
orientation (lhsT = x chunk; bias folded via a rank-1 ones matmul).  The FFN
stays bf16 (fp8 there fails the 2e-2 gate): GEMM1 for the head-0 rows is
interleaved into the attention tail; GEMM2 is orientation-swapped (out =
[row, d], hid as lhsT, b2 via rank-1 fold) so the residual add uses h1
directly and the output DMAs row-major with no transposes back.

Shapes (hardcoded): x (1, 2048, 1024); Wq/Wk/Wv (1024, 1024); W1 (4096,
1024); W2 (1024, 4096); H=16 heads * HD=64; KB=16 kv blocks of 128.
"""

import sys
from contextlib import ExitStack

import numpy as np

for _p in ("/opt/trn_rl_repo", "/root/.axon_site/_ro/trn_rl_repo"):
    if _p not in sys.path:
        sys.path.append(_p)

import concourse.bass as bass  # noqa: E402
import concourse.tile as tile  # noqa: E402
from concourse import bacc, mybir  # noqa: E402
from concourse._compat import with_exitstack  # noqa: E402
from concourse.bass import ds  # noqa: E402
from concourse.bass_utils import run_bass_kernel_spmd  # noqa: E402
from concourse.masks import make_identity  # noqa: E402

D = 1024
H = 16
HD = 64
FF = 4096
N = 2048
KB = 16
NCORES = 8
RQ = N // NCORES  # 256 local rows
P = 128


F32 = mybir.dt.float32
F32R = mybir.dt.float32r
BF16 = mybir.dt.bfloat16
FP8 = mybir.dt.float8e4
DR = mybir.MatmulPerfMode.DoubleRow
WSC = 32.0  # fp8 weight prescale (values ~0.02 -> ~0.64, mid e4m3 range)
AX = mybir.AxisListType
AF = mybir.ActivationFunctionType


@with_exitstack
def _tile_kernel(ctx: ExitStack, tc: tile.TileContext, io: dict):
    nc = tc.nc

    consts = ctx.enter_context(tc.tile_pool(name="consts", bufs=1))
    ident = consts.tile([P, P], F32)
    make_identity(nc, ident)
    identr = consts.tile([P, P], F32R)
    nc.scalar.activation(out=identr, in_=ident, func=AF.Copy)
    identb = consts.tile([P, P], BF16)
    nc.scalar.activation(out=identb, in_=ident, func=AF.Copy)
    ball = consts.tile([P, 60], F32)
    nc.sync.dma_start(out=ball, in_=io["biases"])
    bqp = ball[:, 0:8]  # bq column per g-pair psum layout
    bk1 = ball[:, 8:9]  # bk column in K-proj psum layout
    b1s = ball[:, 20:52]
    ones1 = consts.tile([1, P], BF16)
    nc.gpsimd.memset(ones1, 1.0)
    b2row = consts.tile([1, D], BF16)
    nc.sync.dma_start(out=b2row, in_=io["b2row"])

    persist = ctx.enter_context(tc.tile_pool(name="persist", bufs=1))
    xl = persist.tile([P, 2, D], F32)  # local x rows (residual), [Q, h, d]
    h1 = persist.tile([P, 2, D], F32)
    h1Tb = persist.tile([P, 8, RQ], BF16)  # bf16 copy for GEMM1 rhs
    hid = persist.tile([P, 32, RQ], BF16)  # FFN hidden (GEMM1 out)
    with tc.tile_pool(name="kvq", bufs=1) as kvp:
        # fp8 DoubleRow tiles.  kT8/qT8 rows 0-31: d-half t of k/q channels
        # (contraction tile t covers d in [t*32, t*32+32)); rows 32-39:
        # per-kv-block indicator / -blockmax rows, block K = t*8 + (row-32).
        kT8 = kvp.tile([40, 2, 2, N], FP8)  # [row, t, h, kc]
        qT8 = kvp.tile([40, 2, 2, 16, P], FP8)  # [row, t, h, g, Q]
        # f padded 65->80 so the DoubleRow k-tile stride (2*80 B) is 16B-aligned
        vaug = kvp.tile([P, KB, 2, 80], FP8)
        nm = kvp.tile([P, 2, 16, KB], BF16)  # -max per [Q, h, g, K]
        # kT8[32+r, t, h, kc] = 1 iff (t*8 + r) == kc//128 — static block
        # indicator rows, streamed from DRAM (64KB, built host-side)
        nc.sync.dma_start(out=kT8[32:40, :, :, :], in_=io["ind8"])
        with tc.tile_pool(name="wstream", bufs=5) as wsp:
            h1p = (ident, xl, h1, h1Tb)
            wtiles = []
            _attention(
                tc, io, kvp, identr, identb, bqp, bk1, kT8, vaug, qT8,
                nm, wsp, h1p, wtiles, b1s, hid,
            )
            _ffn_phase(
                tc, io, b1s, ones1, b2row, h1, h1Tb, wsp, wtiles, hid
            )


def _kvq_proj(tc, io, kvp, identr, bk1, kT8, vaug, chunk_hook, psprj):
    # Writes kT8 d-rows 0-31 (both t halves) as fp8; v goes through vT2 +
    # PE transposes into vaug (fp8).  chunk_hook(c) emits pass-A units for
    # the kc range of chunk c.
    nc = tc.nc
    NCH = 4
    CW = N // NCH  # 512
    ones32 = kvp.tile([P, KB * 2], F32)
    nc.gpsimd.memset(ones32, 1.0)
    # den ones column at f=64 of every (K, h) slot
    nc.scalar.activation(
        out=vaug[:, :, :, HD : HD + 1].rearrange("p a b c -> p (a b c)"),
        in_=ones32,
        func=AF.Copy,
    )

    # K/V projections, fp8 DoubleRow (x and weights fp8, weights scaled
    # x32; the 1/32 rides the psum copy-out).  V is computed directly in
    # kc-major orientation (lhsT = x chunk) so no transposes are needed;
    # its bias is folded in through a rank-1 ones matmul.
    pskv = psprj
    with (
        tc.tile_pool(name="wkv", bufs=1) as wp,
        tc.tile_pool(name="xs", bufs=2) as xsp,
    ):
        wk = wp.tile([P, 4, 2, P], FP8)
        wv = wp.tile([P, 4, 2, P], FP8)
        onesn = wp.tile([1, P], FP8)
        nc.gpsimd.memset(onesn, 1.0)
        bv32 = wp.tile([1, P], FP8)
        nc.sync.dma_start(out=bv32, in_=io["bv32"])
        nc.sync.dma_start(
            out=wk, in_=io["wkT"].rearrange("(j t p) c -> p j t c", p=P, t=2)
        )
        nc.sync.dma_start(
            out=wv, in_=io["wvT"].rearrange("(j t p) c -> p j t c", p=P, t=2)
        )
        xTr = io["xT"].rearrange("(j t p) n -> p j t n", p=P, t=2)
        with tc.psum_pool(name="ps_vt", bufs=1) as psvt:
            for c in range(NCH):
                xc = xsp.tile([P, 4, 2, CW], FP8, tag="xc")
                nc.sync.dma_start(out=xc, in_=xTr[:, :, :, ds(c * CW, CW)])
                psk = pskv.tile([P, CW], F32, tag="pkv")
                for j in range(4):
                    nc.tensor.matmul(
                        psk,
                        lhsT=wk[:, j, :, :],
                        rhs=xc[:, j, :, :],
                        start=(j == 0),
                        stop=(j == 3),
                        perf_mode=DR,
                    )
                for h in range(2):
                    for t in range(2):
                        src = psk[ds(h * HD + t * 32, 32), :]
                        if t == 0:
                            nc.scalar.activation(
                                out=kT8[0:32, t, h, ds(c * CW, CW)],
                                in_=src, func=AF.Identity,
                                scale=1.0 / WSC,
                                bias=bk1[ds(h * HD + t * 32, 32), :],
                            )
                        else:
                            nc.vector.tensor_scalar(
                                out=kT8[0:32, t, h, ds(c * CW, CW)],
                                in0=src, scalar1=1.0 / WSC,
                                scalar2=bk1[ds(h * HD + t * 32, 32), :],
                                op0=mybir.AluOpType.mult,
                                op1=mybir.AluOpType.add,
                            )
                # V for the 4 kv blocks of this chunk, kc-major directly
                for K in range(4 * c, 4 * c + 4):
                    pv = psvt.tile([P, P], F32, tag="vt")
                    for j in range(4):
                        nc.tensor.matmul(
                            pv,
                            lhsT=xc[:, j, :, ds((K % 4) * P, P)],
                            rhs=wv[:, j, :, :],
                            start=(j == 0),
                            stop=False,
                            perf_mode=DR,
                        )
                    nc.tensor.matmul(
                        pv, lhsT=onesn, rhs=bv32, start=False, stop=True
                    )
                    nc.scalar.activation(
                        out=vaug[:, K, :, 0:HD],
                        in_=pv.rearrange("p (a b) -> p a b", a=2),
                        func=AF.Identity,
                        scale=1.0 / WSC,
                    )
                chunk_hook(c)

def _attention(
    tc, io, kvp, identr, identb, bqp, bk1, kT8, vaug, qT8, nm,
    wsp, h1p, wtiles, b1s, hid,
):
    """Pass A (q-major scores -> per-block -max, DVE-bound) software-pipelined
    under pass B (kc-major scores+max-broadcast in ONE matmul via the
    indicator rows stacked at partitions 64-79, then exp -> num/den matmul).

    Emission order interleaves A-units (one g at a time) ahead of the B
    chunks that consume their -max rows, so DVE reductions hide under PE.
    The Q projection is fused in front, with the first 8 A-units interleaved
    so DVE starts early; h1/h1T for each head are built as soon as its
    attention output finalizes.
    """
    nc = tc.nc
    ident, xl, h1, h1Tb = h1p

    with (
        tc.tile_pool(name="et", bufs=3) as etp,
        tc.tile_pool(name="fin", bufs=3) as finp,
        tc.psum_pool(name="ps_t", bufs=1) as pst,
        tc.psum_pool(name="ps_A", bufs=2) as psa,
    ):

        def a_mm_part(h, g, sh):
            # q-major scores for (h, g), kc chunk sh; -max per block.
            # Reduce engine alternates DVE/Pool to split the 84us of max work.
            s = psa.tile([P, 4 * P], F32, tag="s")
            nc.tensor.matmul(
                s,
                lhsT=qT8[0:32, :, h, g, :],
                rhs=kT8[0:32, :, h, ds(sh * 512, 512)],
                start=True,
                stop=True,
                perf_mode=DR,
            )
            # -max over the even kc of each block (stride-2 sample halves the
            # DVE read volume; the block-scale tolerance absorbs the bias)
            nc.vector.reduce_max(
                out=nm[:, h, g, ds(sh * 4, 4)],
                in_=s.rearrange("p (b f t) -> p b f t", f=HD, t=2)[:, :, :, 0],
                axis=AX.X,
                negate=True,
            )

        def a_mm(h, g):
            for sh in range(4):
                a_mm_part(h, g, sh)

        def a_nt(h, qc):
            # -max rows into qT8 partitions 32-39 for a whole g-quad: eight
            # small transposes land every (t, g) half in one 8-partition psum
            # tile, then ONE copy fills all t/g slots of qT8 for this chunk.
            nt = pst.tile([8, 2, 4, P], BF16, tag="t")
            for j in range(4):
                for t in range(2):
                    nc.tensor.transpose(
                        nt[:, t, j, :],
                        nm[:, h, qc * 4 + j, ds(t * 8, 8)],
                        identb,
                    )
            if h == 0:
                nc.scalar.activation(
                    out=qT8[32:40, :, h, ds(qc * 4, 4), :], in_=nt,
                    func=AF.Copy,
                )
            else:
                nc.vector.tensor_copy(
                    out=qT8[32:40, :, h, ds(qc * 4, 4), :], in_=nt
                )

        nonlocal_pools = {}

        def b_chunk(h, qc):
            # 512 q-things (4 g), all kv blocks in pairs; one exp per pair;
            # st pairs emitted ahead of num so PE never waits on ACT.
            psst = nonlocal_pools["psst"]
            psn = nonlocal_pools["psn"]
            nacc = psn.tile([HD + 1, 512], F32, tag="nacc")
            ets = []

            def mm_st(p):
                st = psst.tile([P, 2, 512], F32, tag="st")
                for i in range(2):
                    nc.tensor.matmul(
                        st[:, i, :],
                        lhsT=kT8[:, :, h, ds((2 * p + i) * P, P)],
                        rhs=qT8[:, :, h, ds(qc * 4, 4), :],
                        start=True,
                        stop=True,
                        perf_mode=DR,
                    )
                et = etp.tile([P, 2, 512], FP8, tag="et")
                nc.scalar.activation(out=et, in_=st, func=AF.Exp, scale=0.125)
                ets.append(et)

            def mm_num(p):
                nc.tensor.matmul(
                    nacc,
                    lhsT=vaug[:, 2 * p : 2 * p + 2, h, 0 : HD + 1],
                    rhs=ets[p],
                    start=(p == 0),
                    stop=(p == KB // 2 - 1),
                    perf_mode=DR,
                )

            NP2 = KB // 2
            mm_st(0)
            mm_st(1)
            for p in range(NP2):
                if p + 2 < NP2:
                    mm_st(p + 2)
                mm_num(p)
            if h == 0:
                nc.scalar.activation(
                    out=nsb[:, h, ds(qc * 512, 512)], in_=nacc,
                    func=AF.Identity,
                )
            else:
                nc.vector.tensor_copy(
                    out=nsb[:, h, ds(qc * 512, 512)], in_=nacc
                )

        def fin_quad(h, qc):
            # transpose num/den back to Q-partitions for the whole g-quad,
            # one batched reciprocal, then per-g fused stt:
            # h1 = num*(1/den) + x  (residual folded in; no attn tile)
            tr = pst.tile([P, 4, 80], BF16, tag="t")
            for j in range(4):
                nc.tensor.transpose(
                    tr[:, j, 0 : HD + 1],
                    nsb[:, h, ds((qc * 4 + j) * P, P)],
                    identb[0 : HD + 1, 0 : HD + 1],
                )
            rcp4 = finp.tile([P, 4], F32, tag="rcp")
            nc.vector.reciprocal(out=rcp4, in_=tr[:, :, HD])
            for j in range(4):
                g = qc * 4 + j
                nc.vector.scalar_tensor_tensor(
                    out=h1[:, h, ds(g * HD, HD)],
                    in0=tr[:, j, 0:HD],
                    scalar=rcp4[:, j : j + 1],
                    in1=xl[:, h, ds(g * HD, HD)],
                    op0=mybir.AluOpType.mult,
                    op1=mybir.AluOpType.add,
                )

        def g1h0(ffpair):
            # GEMM1 for head-0 rows (0:128), ff blocks ffpair*2..+2, on the
            # attention tail's idle PE; h0's h1Tb is complete after chunk 3.
            s = psa.tile([P, 4 * P], F32, tag="s")
            for fi in range(2):
                ff = ffpair * 2 + fi
                w1q = wtiles[ff // 8]
                for dc in range(8):
                    nc.tensor.matmul(
                        s[:, ds(fi * P, P)],
                        lhsT=w1q[:, dc, ds((ff % 8) * P, P)],
                        rhs=h1Tb[:, dc, 0:P],
                        start=(dc == 0),
                        stop=(dc == 7),
                    )
            for fi in range(2):
                ff = ffpair * 2 + fi
                if ff % 2 == 0:
                    nc.scalar.activation(
                        out=hid[:, ff, 0:P], in_=s[:, ds(fi * P, P)],
                        func=AF.Relu, bias=b1s[:, ff : ff + 1],
                    )
                else:
                    nc.vector.tensor_scalar(
                        out=hid[:, ff, 0:P], in0=s[:, ds(fi * P, P)],
                        scalar1=b1s[:, ff : ff + 1], scalar2=0.0,
                        op0=mybir.AluOpType.add, op1=mybir.AluOpType.max,
                    )

        def h1_seg(h, qc):
            # h1 (already includes the residual via fin_quad's stt); build
            # the two h1Tb (bf16, d-major) column blocks GEMM1 needs as rhs
            # with two transposes into one psum tile and a single copy
            tps = pst.tile([P, 2, P], F32, tag="t")
            for i, dc in enumerate((qc * 2, qc * 2 + 1)):
                nc.tensor.transpose(tps[:, i, :], h1[:, h, ds(dc * P, P)], ident)
            dst = h1Tb[:, ds(qc * 2, 2), ds(h * P, P)]
            if h == 0:
                nc.scalar.activation(out=dst, in_=tps, func=AF.Copy)
            else:
                nc.vector.tensor_copy(out=dst, in_=tps)

        # Q projection (g-pairs; dout 128 at a time), pre-scaled 1/8
        # host-side.  One psum pool (ps_prj) spans the Q and KV phases.
        psprj_cm = tc.psum_pool(name="ps_prj", bufs=2)
        psprj = psprj_cm.__enter__()
        with (
            tc.tile_pool(name="wq", bufs=2) as wqp,
            tc.tile_pool(name="xtl", bufs=1) as xtlp,
        ):
            xtl = xtlp.tile([P, 4, 2, RQ], FP8)
            nc.sync.dma_start(
                out=xtl,
                in_=io["xTloc"].rearrange("(j t p) n -> p j t n", p=P, t=2),
            )
            wqr = io["wqT"].rearrange("(j t p) c -> p j t c", p=P, t=2)
            for half in range(2):
                wq = wqp.tile([P, 4, 2, 4 * P], FP8, tag="wq")
                nc.sync.dma_start(
                    out=wq, in_=wqr[:, :, :, ds(half * 4 * P, 4 * P)]
                )
                for tq in range(4):
                    gp = half * 4 + tq  # g-pair index; g = 2*gp, 2*gp+1
                    ps = psprj.tile([P, RQ], F32, tag="pq")
                    for j in range(4):
                        nc.tensor.matmul(
                            ps,
                            lhsT=wq[:, j, :, ds(tq * P, P)],
                            rhs=xtl[:, j, :, :],
                            start=(j == 0),
                            stop=(j == 3),
                            perf_mode=DR,
                        )
                    for gh in range(2):
                        g = 2 * gp + gh
                        for t in range(2):
                            src = ps[ds(gh * HD + t * 32, 32), :].rearrange(
                                "p (h q) -> p h q", h=2
                            )
                            bia = bqp[ds(gh * HD + t * 32, 32), gp : gp + 1]
                            if t == 0:
                                nc.scalar.activation(
                                    out=qT8[0:32, t, :, g, :], in_=src,
                                    func=AF.Identity, scale=1.0 / WSC,
                                    bias=bia,
                                )
                            else:
                                nc.vector.tensor_scalar(
                                    out=qT8[0:32, t, :, g, :], in0=src,
                                    scalar1=1.0 / WSC, scalar2=bia,
                                    op0=mybir.AluOpType.mult,
                                    op1=mybir.AluOpType.add,
                                )

        # KV projection, with pass-A units for the first two chunks'
        # g-sets pipelined into each kc chunk (fills the DMA-bound
        # projection window with DVE work)
        _kvq_proj(
            tc, io, kvp, identr, bk1, kT8, vaug,
            lambda c: [a_mm_part(0, g, c) for g in range(8)],
            psprj,
        )
        psprj_cm.__exit__(None, None, None)
        nsb = kvp.tile([HD + 1, 2, N], BF16)  # [f(+den), h, (g,Q)]

        # chunk c consumes g-set G(c); a_mm runs 2 chunks ahead (G0/G1 were
        # emitted in the KV hook), a_nt 1 ahead, fin one behind.  Heads
        # alternate so the fin/h1 engine split (ACT for h0, DVE for h1)
        # stays balanced across the loop.
        chunks = [(h, qc) for h in range(2) for qc in range(4)]
        gsets = [[(h, qc * 4 + j) for j in range(4)] for h, qc in chunks]
        with (
            tc.psum_pool(name="ps_st", bufs=2) as psst_,
            tc.psum_pool(name="ps_n", bufs=1) as psn_,
        ):
            nonlocal_pools["psst"] = psst_
            nonlocal_pools["psn"] = psn_
            a_nt(*chunks[0])
            for c, (h, qc) in enumerate(chunks):
                if c == 0:
                    nc.sync.dma_start(out=xl[:, 0, :], in_=io["xloc"][0:P, :])
                    nc.sync.dma_start(
                        out=xl[:, 1, :], in_=io["xloc"][P : 2 * P, :]
                    )
                if c == 1:
                    # FFN weight chunks stream while DMA engines idle; later
                    # chunks are requested as g1h0 frees wstream slots
                    wtiles.append(_ffn_weight_dma(tc, io, wsp, 0))
                    wtiles.append(_ffn_weight_dma(tc, io, wsp, 1))
                if c == 3:
                    wtiles.append(_ffn_weight_dma(tc, io, wsp, 2))
                for wc in {4: [3], 5: [4], 6: [5, 6], 7: [7]}.get(c, []):
                    wtiles.append(_ffn_weight_dma(tc, io, wsp, wc))
                if c + 2 < len(chunks):
                    for hg in gsets[c + 2]:
                        a_mm(*hg)
                if c + 1 < len(chunks):
                    a_nt(*chunks[c + 1])
                b_chunk(h, qc)
                if c > 0:
                    ph, pqc = chunks[c - 1]
                    fin_quad(ph, pqc)
                    h1_seg(ph, pqc)
                if c >= 4:
                    for fp in range(4 * (c - 4), 4 * (c - 4) + 4):
                        g1h0(fp)
            ph, pqc = chunks[-1]
            fin_quad(ph, pqc)
            h1_seg(ph, pqc)


def _ffn_weight_dma(tc, io, wsp, chunk):
    # one batched DMA per 1024-col bf16 weight chunk; chunks 0-3 = W1
    # quarters, 4-7 = W2 quarters
    nc = tc.nc
    w = wsp.tile([P, 8, 8 * P], BF16, tag="wbig")
    if chunk < 4:
        src = io["w1T"].rearrange("(j p) c -> p j c", p=P)[
            :, :, ds(chunk * 8 * P, 8 * P)
        ]
    else:
        q2 = chunk - 4
        src = io["w2T"][ds(q2 * 8 * P, 8 * P), :].rearrange(
            "(j p) c -> p j c", p=P
        )
    nc.sync.dma_start(out=w, in_=src)
    return w


def _ffn_phase(tc, io, b1s, ones1, b2row, h1, h1Tb, wsp, wtiles, hid):
    nc = tc.nc
    with (
        tc.tile_pool(name="ffn_sm", bufs=3) as fsm,
    ):
        with tc.psum_pool(name="ps_f", bufs=3) as psf:
            for q4 in range(4):  # W1 column quarters [128, 8, 1024]
                w1q = wtiles[q4]
                # all ff head-0 rows were filled during the attention tail
                cs = ds(P, P)
                for f in range(8):
                    ff = q4 * 8 + f
                    ps = psf.tile([P, RQ], F32, tag="fps")
                    for dc in range(8):
                        nc.tensor.matmul(
                            ps[:, cs],
                            lhsT=w1q[:, dc, ds(f * P, P)],
                            rhs=h1Tb[:, dc, cs],
                            start=(dc == 0),
                            stop=(dc == 7),
                        )
                    nc.scalar.activation(
                        out=hid[:, ff, cs], in_=ps[:, cs], func=AF.Relu,
                        bias=b1s[:, ff : ff + 1],
                    )
        # GEMM2 swapped: out [row, d]; contraction over ff via hid as lhsT.
        # b2 folded in through a rank-1 ones matmul; residual add uses h1
        # directly (same orientation), so no transposes back.
        with tc.psum_pool(name="ps_y", bufs=1) as psy:
            yps = [
                psy.tile([P, 512], F32, tag=f"yp{j}", name=f"yp{j}")
                for j in range(4)
            ]
            for q2 in range(4):
                w2q = wtiles[4 + q2]
                for h in range(2):
                    for dc2 in range(2):
                        yp = yps[h * 2 + dc2]
                        for fc in range(8):
                            nc.tensor.matmul(
                                yp,
                                lhsT=hid[:, q2 * 8 + fc, ds(h * P, P)],
                                rhs=w2q[:, fc, ds(dc2 * 512, 512)],
                                start=(q2 == 0 and fc == 0),
                                stop=False,
                            )
            for h in range(2):
                for dc2 in range(2):
                    yp = yps[h * 2 + dc2]
                    nc.tensor.matmul(
                        yp,
                        lhsT=ones1,
                        rhs=b2row[0:1, ds(dc2 * 512, 512)],
                        start=False,
                        stop=True,
                    )
                    osb = fsm.tile([P, 512], F32, tag="osb")
                    nc.vector.tensor_add(
                        out=osb, in0=yp, in1=h1[:, h, ds(dc2 * 512, 512)]
                    )
                    nc.sync.dma_start(
                        out=io["outR"][ds(h * P, P), ds(dc2 * 512, 512)],
                        in_=osb,
                    )


def _build(reps=1):
    nc = bacc.Bacc(
        "TRN2", target_bir_lowering=False, debug=False, num_devices=NCORES
    )
    io = {}
    def inp(name, shape, dt=F32):
        io[name] = nc.dram_tensor(name, shape, dt, kind="ExternalInput").ap()
    inp("xT", [D, N], FP8)
    inp("xTloc", [D, RQ], FP8)
    inp("xloc", [RQ, D])
    inp("wqT", [D, D], FP8)
    inp("wkT", [D, P], FP8)
    inp("wvT", [D, P], FP8)
    inp("bv32", [1, P], FP8)
    inp("ind8", [8, 2, 2, N], FP8)
    inp("w1T", [D, FF], BF16)
    inp("w2T", [FF, D], BF16)
    inp("biases", [P, 60])
    inp("b2row", [1, D], BF16)
    io["outR"] = nc.dram_tensor("outR", [RQ, D], F32, kind="ExternalOutput").ap()
    with tile.TileContext(nc) as tc:
        for _ in range(reps):
            _tile_kernel(tc, io)
    nc.compile()
    return nc


_CACHE = {}


def _get_nc():
    if "nc" not in _CACHE:
        _CACHE["nc"] = _build()
    return _CACHE["nc"]


_BF16NP = mybir.dt.np(BF16)
_FP8NP = mybir.dt.np(FP8)


def _rows_for_core(c):
    # local row r = h*128 + Q  ->  global n = Q*16 + 2c + h
    r = np.arange(RQ)
    h, Q = r // P, r % P
    return Q * 16 + 2 * c + h


def make_in_maps(inputs):
    x = np.ascontiguousarray(np.asarray(inputs["x"], np.float32)[0])
    xT = np.ascontiguousarray(x.T)

    wqT8 = np.ascontiguousarray(np.asarray(inputs["Wq"], np.float32).T)
    bq8 = np.asarray(inputs["bq"], np.float32)
    wkT = np.asarray(inputs["Wk"], np.float32).T  # [din, dout]
    wvT = np.asarray(inputs["Wv"], np.float32).T
    bk = np.asarray(inputs["bk"], np.float32)
    bv = np.asarray(inputs["bv"], np.float32)

    def b2d(b, k):
        return np.asarray(b, np.float32).reshape(k, P).T

    ind8 = np.zeros((8, 2, 2, N), np.float32)
    for r in range(8):
        for t in range(2):
            K = t * 8 + r
            ind8[r, t, :, K * P : (K + 1) * P] = 1.0
    common = {
        "ind8": ind8.astype(_FP8NP),
        "xT": xT.astype(_FP8NP),
        "wqT": (wqT8 * WSC).astype(_FP8NP),
        "w1T": np.ascontiguousarray(
            np.asarray(inputs["W1"], np.float32).T.astype(_BF16NP)
        ),
        "w2T": np.ascontiguousarray(
            np.asarray(inputs["W2"], np.float32).T.astype(_BF16NP)
        ),
    }
    in_maps = []
    for c in range(NCORES):
        rows = _rows_for_core(c)
        sl = slice(c * P, (c + 1) * P)
        m = dict(common)
        m["xTloc"] = np.ascontiguousarray(xT[:, rows]).astype(_FP8NP)
        m["xloc"] = np.ascontiguousarray(x[rows])
        m["wkT"] = np.ascontiguousarray(wkT[:, sl] * WSC).astype(_FP8NP)
        m["wvT"] = np.ascontiguousarray(wvT[:, sl] * WSC).astype(_FP8NP)
        m["bv32"] = (bv[sl] * WSC).reshape(1, P).astype(_FP8NP)
        ball = np.zeros((P, 60), np.float32)
        ball[:, 0:8] = bq8.reshape(8, P).T  # column = g-pair psum layout
        ball[:, 8] = bk[sl]
        ball[:, 20:52] = b2d(inputs["b1"], 32)
        m["biases"] = ball
        m["b2row"] = np.asarray(inputs["b2"], np.float32).reshape(1, D).astype(
            _BF16NP
        )
        in_maps.append(m)
    return in_maps


def kernel(**inputs):
    nc = _get_nc()
    res = run_bass_kernel_spmd(nc, make_in_maps(inputs), core_ids=list(range(NCORES)))
    out = np.empty((1, N, D), np.float32)
    for c in range(NCORES):
        out[0, _rows_for_core(c), :] = res.results[c]["outR"]
    return out

